# revision 23
# baseline (speedup 1.0000x reference)
"""VariableSelectionNetwork Trainium2 kernel (8-core data parallel).

Reference computation per row n (F=32 features, H=64 hidden):
    t[n,f,h] = feat[n,f]*W_feat[f,h] + b_feat[f,h]
    gates    = softmax(t.flat @ W_gate + b_gate)        # over f
    out[n,h] = sum_f t[n,f,h] * gates[n,f]

Algebraic collapse (exact, just reassociated):
    logits = feat @ A + c        A[f,g] = sum_h W_feat[f,h]*W_gate[f*H+h,g]
                                 c      = b_feat.flat @ W_gate + b_gate
    e      = exp(logits)         (logits are O(1); no max-shift needed)
    gates  = e * (1 / sum_f e)
    out    = (feat*gates) @ W_feat + gates @ b_feat

Device dataflow per core (2048 rows), "blocked transpose" scheme, 2 slabs:
    ftile[p, (rb f)] <- DMA          (row = p*16+rb; contiguous per partition)
    featT = StreamTranspose(ftile)   -> featT[(pb f), (rb pl)], row=(pb*32+pl)*16+rb
    lg    = blockdiag(A,x4).T @ featT            (one matmul per slab)
    et    = exp(lg + c_rep)                      (ACT, bias per partition)
    dsum  = blockdiag(ones32,x4).T @ et          -> [4, n] row sums over f
    rcp   = reciprocal_approx_fast(dsum)         (DVE custom op)
    rcp_b = gpsimd partition_broadcast per pb    -> [128, n]
    gatesT= et * rcp_b ; wgfT = featT * gatesT   (DVE)
    outT_hb = blockdiag(W_feat[:,hb],x4).T @ wgfT + blockdiag(b_feat[:,hb],x4).T @ gatesT
    otile[p, (rb h)] = StreamTranspose(outT)     -> DMA out
"""

import sys

sys.path.insert(0, "/opt/trn_rl_repo")

import numpy as np

from concourse import bacc, mybir, tile
from concourse.bass_utils import run_bass_kernel_spmd

B, S, F, H = 32, 512, 32, 64
N = B * S
NCORES = 8
NC_ROWS = N // NCORES  # 2048 rows per core
P = 128
NPB = P // F           # 4 partition blocks
RPP = NC_ROWS // P     # 16 rows per partition
NS = 2                 # slabs (pipeline stages over rb)
RBS = RPP // NS        # rb per slab
NW = RBS * F           # transposed-domain columns per slab
F32 = mybir.dt.float32
F32R = mybir.dt.float32r
EXP = mybir.ActivationFunctionType.Exp
MMDT = F32R            # dtype for matmul operands (f32r: 1-pass PE at N>=256)

# packed param columns: pp1 = [bdA | crep] (needed first), pp2 = rest
PW1 = P + 1
PW2 = 5 * P
_C_BDW = [0, 2 * P]
_C_BDB = [P, 3 * P]
_C_BD1 = 4 * P

_NC_CACHE = {}


def _build_nc():
    nc = bacc.Bacc("TRN2", target_bir_lowering=False, debug=False, num_devices=NCORES)

    feat_d = nc.dram_tensor("feat", [NC_ROWS, F], F32, kind="ExternalInput").ap()
    pp1_d = nc.dram_tensor("pp1", [P, PW1], MMDT, kind="ExternalInput").ap()
    pp2_d = nc.dram_tensor("pp2", [P, PW2], MMDT, kind="ExternalInput").ap()
    out_d = nc.dram_tensor("out", [NC_ROWS, H], F32, kind="ExternalOutput").ap()

    with tile.TileContext(nc) as tc:
        with (
            tc.tile_pool(name="const", bufs=1) as cpool,
            tc.tile_pool(name="work", bufs=2) as work,
            tc.tile_pool(name="ps", bufs=2, space="PSUM") as ps,
        ):
            pp1 = cpool.tile([P, PW1], MMDT)
            nc.scalar.dma_start(pp1[:], pp1_d)

            feat_r0 = feat_d.rearrange("(p r) f -> p (r f)", p=P)
            ftiles = []
            for s in range(NS):
                ld_eng = nc.sync if s % 2 == 0 else nc.scalar
                ft = work.tile([P, NW], F32, tag="fin")
                ld_eng.dma_start(ft[:], feat_r0[:, s * NW : (s + 1) * NW])
                ftiles.append(ft)

            pp2 = cpool.tile([P, PW2], MMDT)
            nc.sync.dma_start(pp2[:], pp2_d)
            bda = pp1[:, 0:P]
            crep = pp1[:, P : P + 1]
            bdw = [pp2[:, c : c + P] for c in _C_BDW]
            bdb = [pp2[:, c : c + P] for c in _C_BDB]
            bd1x = pp2[:, _C_BD1 : _C_BD1 + P]

            feat_r = feat_d.rearrange("(p r) f -> p (r f)", p=P)   # [128, 512]
            out_r = out_d.rearrange("(p r) h -> p (r h)", p=P)     # [128, 1024]

            from copy import deepcopy

            for s in range(NS):
                st_eng = nc.scalar if s % 2 == 0 else nc.sync
                ftile = ftiles[s]

                featT = work.tile([P, NW], F32, tag="featT")
                nc.vector.transpose(featT[:], ftile[:])
                featTr = work.tile([P, NW], MMDT, tag="featTr")
                nc.scalar.copy(featTr[:], featT[:])

                lg = ps.tile([P, NW], F32, tag="lg")
                nc.tensor.matmul(lg[:], bda, featTr[:])

                et = work.tile([P, NW], MMDT, tag="et")
                nc.scalar.activation(et[:], lg[:], EXP, bias=crep)

                # unnormalized: wef = featT*e feeds the out matmuls directly;
                # the reciprocal runs in parallel off the critical chain
                dsum_b = ps.tile([P, NW], F32, tag="dsum")
                nc.tensor.matmul(dsum_b[:], bd1x, et[:])
                rcp_b = work.tile([P, NW], F32, tag="rcpb")
                nc.vector.reciprocal_approx_fast(rcp_b[:], dsum_b[:])

                wef = work.tile([P, NW], MMDT, tag="wef")
                nc.vector.tensor_mul(wef[:], featTr[:], et[:])

                outT = ps.tile([P, 2 * NW], F32, tag="outT")
                for hb in range(2):
                    osl = outT[:, hb * NW : (hb + 1) * NW]
                    nc.tensor.matmul(osl, bdw[hb], wef[:], start=True, stop=False)
                    nc.tensor.matmul(osl, bdb[hb], et[:], start=False, stop=True)

                # normalize both h-halves with one broadcast mul
                outTn = work.tile([P, 2 * NW], F32, tag="outTn")
                rcp_bc = rcp_b[:].rearrange("p (o n) -> p o n", o=1)
                rcp_bc = deepcopy(rcp_bc)
                rcp_bc.ap[1] = [0, 2]
                nc.vector.tensor_mul(
                    outTn[:].rearrange("p (o n) -> p o n", o=2),
                    outT[:].rearrange("p (o n) -> p o n", o=2),
                    rcp_bc,
                )

                otile = work.tile([P, RBS * H], F32, tag="oout")
                otile_blk = otile[:].rearrange("p (rb z) -> p rb z", rb=RBS)
                for hb in range(2):
                    nc.vector.transpose(
                        otile_blk[:, :, hb * F : (hb + 1) * F],
                        outTn[:, hb * NW : (hb + 1) * NW],
                    )
                st_eng.dma_start(
                    out_r[:, s * RBS * H : (s + 1) * RBS * H], otile[:]
                )

    nc.compile()
    return nc


def _get_nc():
    if "nc" not in _NC_CACHE:
        _NC_CACHE["nc"] = _build_nc()
    return _NC_CACHE["nc"]


def _prep_params(W_feat, b_feat, W_gate, b_gate):
    wf = np.asarray(W_feat, np.float64)
    wg = np.asarray(W_gate, np.float64).reshape(F, H, F)
    a = np.einsum("fh,fhg->fg", wf, wg).astype(np.float32)
    c = (
        np.asarray(b_feat, np.float64).reshape(-1) @ np.asarray(W_gate, np.float64)
        + np.asarray(b_gate, np.float64)
    ).astype(np.float32)
    wf32 = np.asarray(W_feat, np.float32)
    bf32 = np.asarray(b_feat, np.float32)

    pp1 = np.zeros((P, PW1), np.float32)
    pp2 = np.zeros((P, PW2), np.float32)
    for pb in range(NPB):
        sl = slice(pb * F, (pb + 1) * F)
        pp1[sl, pb * F : (pb + 1) * F] = a
        pp1[sl, P] = c
        for hb in range(2):
            pp2[sl, _C_BDW[hb] + pb * F : _C_BDW[hb] + (pb + 1) * F] = (
                wf32[:, hb * F : (hb + 1) * F]
            )
            pp2[sl, _C_BDB[hb] + pb * F : _C_BDB[hb] + (pb + 1) * F] = (
                bf32[:, hb * F : (hb + 1) * F]
            )
        pp2[sl, _C_BD1 + pb * F : _C_BD1 + (pb + 1) * F] = 1.0
    return {"pp1": pp1, "pp2": pp2}


def kernel(features, W_feat, b_feat, W_gate, b_gate):
    params = _prep_params(W_feat, b_feat, W_gate, b_gate)
    featf = np.ascontiguousarray(np.asarray(features, np.float32).reshape(N, F))
    nc = _get_nc()
    in_maps = [
        {"feat": featf[i * NC_ROWS : (i + 1) * NC_ROWS], **params}
        for i in range(NCORES)
    ]
    res = run_bass_kernel_spmd(nc, in_maps, list(range(NCORES))).results
    out = np.concatenate([res[i]["out"] for i in range(NCORES)], axis=0)
    return out.reshape(B, S, H)


# revision 24
# speedup vs baseline: 1.0472x; 1.0472x over previous
"""VariableSelectionNetwork Trainium2 kernel (8-core data parallel).

Reference computation per row n (F=32 features, H=64 hidden):
    t[n,f,h] = feat[n,f]*W_feat[f,h] + b_feat[f,h]
    gates    = softmax(t.flat @ W_gate + b_gate)        # over f
    out[n,h] = sum_f t[n,f,h] * gates[n,f]

Algebraic collapse (exact, just reassociated):
    logits = feat @ A + c        A[f,g] = sum_h W_feat[f,h]*W_gate[f*H+h,g]
                                 c      = b_feat.flat @ W_gate + b_gate
    e      = exp(logits)         (logits are O(1); no max-shift needed)
    gates  = e * (1 / sum_f e)
    out    = (feat*gates) @ W_feat + gates @ b_feat

Device dataflow per core (2048 rows), "blocked transpose" scheme, 2 slabs:
    ftile[p, (rb f)] <- DMA          (row = p*16+rb; contiguous per partition)
    featT = StreamTranspose(ftile)   -> featT[(pb f), (rb pl)], row=(pb*32+pl)*16+rb
    lg    = blockdiag(A,x4).T @ featT            (one matmul per slab)
    et    = exp(lg + c_rep)                      (ACT, bias per partition)
    dsum  = blockdiag(ones32,x4).T @ et          -> [4, n] row sums over f
    rcp   = reciprocal_approx_fast(dsum)         (DVE custom op)
    rcp_b = gpsimd partition_broadcast per pb    -> [128, n]
    gatesT= et * rcp_b ; wgfT = featT * gatesT   (DVE)
    outT_hb = blockdiag(W_feat[:,hb],x4).T @ wgfT + blockdiag(b_feat[:,hb],x4).T @ gatesT
    otile[p, (rb h)] = StreamTranspose(outT)     -> DMA out
"""

import sys

sys.path.insert(0, "/opt/trn_rl_repo")

import numpy as np

from concourse import bacc, mybir, tile
from concourse.bass_utils import run_bass_kernel_spmd

B, S, F, H = 32, 512, 32, 64
N = B * S
NCORES = 8
NC_ROWS = N // NCORES  # 2048 rows per core
P = 128
NPB = P // F           # 4 partition blocks
RPP = NC_ROWS // P     # 16 rows per partition
NS = 2                 # slabs (pipeline stages over rb)
RBS = RPP // NS        # rb per slab
NW = RBS * F           # transposed-domain columns per slab
F32 = mybir.dt.float32
F32R = mybir.dt.float32r
EXP = mybir.ActivationFunctionType.Exp
MMDT = F32R            # dtype for matmul operands (f32r: 1-pass PE at N>=256)

# packed param columns: pp1 = [bdA | crep] (needed first), pp2 = rest
PW1 = P + 1
PW2 = 5 * P
_C_BDW = [0, 2 * P]
_C_BDB = [P, 3 * P]
_C_BD1 = 4 * P

_NC_CACHE = {}


def _build_nc():
    nc = bacc.Bacc("TRN2", target_bir_lowering=False, debug=False, num_devices=NCORES)

    feat_d = nc.dram_tensor("feat", [NC_ROWS, F], F32, kind="ExternalInput").ap()
    pp1_d = nc.dram_tensor("pp1", [P, PW1], MMDT, kind="ExternalInput").ap()
    pp2_d = nc.dram_tensor("pp2", [P, PW2], MMDT, kind="ExternalInput").ap()
    out_d = nc.dram_tensor("out", [NC_ROWS, H], F32, kind="ExternalOutput").ap()

    with tile.TileContext(nc) as tc:
        with (
            tc.tile_pool(name="const", bufs=1) as cpool,
            tc.tile_pool(name="work", bufs=2) as work,
            tc.tile_pool(name="ps", bufs=2, space="PSUM") as ps,
        ):
            feat_r0 = feat_d.rearrange("(p r) f -> p (r f)", p=P)
            fbig = work.tile([P, NS * NW], F32, tag="fin")
            nc.sync.dma_start(fbig[:], feat_r0)
            ftiles = [fbig[:, s * NW : (s + 1) * NW] for s in range(NS)]

            pp1 = cpool.tile([P, PW1], MMDT)
            nc.scalar.dma_start(pp1[:], pp1_d)
            pp2 = cpool.tile([P, PW2], MMDT)
            nc.scalar.dma_start(pp2[:], pp2_d)
            bda = pp1[:, 0:P]
            crep = pp1[:, P : P + 1]
            bdw = [pp2[:, c : c + P] for c in _C_BDW]
            bdb = [pp2[:, c : c + P] for c in _C_BDB]
            bd1x = pp2[:, _C_BD1 : _C_BD1 + P]

            feat_r = feat_d.rearrange("(p r) f -> p (r f)", p=P)   # [128, 512]
            out_r = out_d.rearrange("(p r) h -> p (r h)", p=P)     # [128, 1024]

            from copy import deepcopy

            for s in range(NS):
                st_eng = nc.scalar if s % 2 == 0 else nc.sync
                featT = work.tile([P, NW], F32, tag="featT")
                nc.vector.transpose(featT[:], ftiles[s])
                featTr = work.tile([P, NW], MMDT, tag="featTr")
                nc.scalar.copy(featTr[:], featT[:])

                lg = ps.tile([P, NW], F32, tag="lg")
                nc.tensor.matmul(lg[:], bda, featTr[:])

                et = work.tile([P, NW], MMDT, tag="et")
                nc.scalar.activation(et[:], lg[:], EXP, bias=crep)

                # unnormalized: wef = featT*e feeds the out matmuls directly;
                # the reciprocal runs in parallel off the critical chain
                dsum_b = ps.tile([P, NW], F32, tag="dsum")
                nc.tensor.matmul(dsum_b[:], bd1x, et[:])
                rcp_b = work.tile([P, NW], F32, tag="rcpb")
                nc.vector.reciprocal_approx_fast(rcp_b[:], dsum_b[:])

                wef = work.tile([P, NW], MMDT, tag="wef")
                nc.vector.tensor_mul(wef[:], featTr[:], et[:])

                outT = ps.tile([P, 2 * NW], F32, tag="outT")
                for hb in range(2):
                    osl = outT[:, hb * NW : (hb + 1) * NW]
                    nc.tensor.matmul(osl, bdw[hb], wef[:], start=True, stop=False)
                    nc.tensor.matmul(osl, bdb[hb], et[:], start=False, stop=True)

                # normalize both h-halves with one broadcast mul
                outTn = work.tile([P, 2 * NW], F32, tag="outTn")
                rcp_bc = rcp_b[:].rearrange("p (o n) -> p o n", o=1)
                rcp_bc = deepcopy(rcp_bc)
                rcp_bc.ap[1] = [0, 2]
                nc.vector.tensor_mul(
                    outTn[:].rearrange("p (o n) -> p o n", o=2),
                    outT[:].rearrange("p (o n) -> p o n", o=2),
                    rcp_bc,
                )

                otile = work.tile([P, RBS * H], F32, tag="oout")
                otile_blk = otile[:].rearrange("p (rb z) -> p rb z", rb=RBS)
                for hb in range(2):
                    nc.vector.transpose(
                        otile_blk[:, :, hb * F : (hb + 1) * F],
                        outTn[:, hb * NW : (hb + 1) * NW],
                    )
                st_eng.dma_start(
                    out_r[:, s * RBS * H : (s + 1) * RBS * H], otile[:]
                )

    nc.compile()
    return nc


def _get_nc():
    if "nc" not in _NC_CACHE:
        _NC_CACHE["nc"] = _build_nc()
    return _NC_CACHE["nc"]


def _prep_params(W_feat, b_feat, W_gate, b_gate):
    wf = np.asarray(W_feat, np.float64)
    wg = np.asarray(W_gate, np.float64).reshape(F, H, F)
    a = np.einsum("fh,fhg->fg", wf, wg).astype(np.float32)
    c = (
        np.asarray(b_feat, np.float64).reshape(-1) @ np.asarray(W_gate, np.float64)
        + np.asarray(b_gate, np.float64)
    ).astype(np.float32)
    wf32 = np.asarray(W_feat, np.float32)
    bf32 = np.asarray(b_feat, np.float32)

    pp1 = np.zeros((P, PW1), np.float32)
    pp2 = np.zeros((P, PW2), np.float32)
    for pb in range(NPB):
        sl = slice(pb * F, (pb + 1) * F)
        pp1[sl, pb * F : (pb + 1) * F] = a
        pp1[sl, P] = c
        for hb in range(2):
            pp2[sl, _C_BDW[hb] + pb * F : _C_BDW[hb] + (pb + 1) * F] = (
                wf32[:, hb * F : (hb + 1) * F]
            )
            pp2[sl, _C_BDB[hb] + pb * F : _C_BDB[hb] + (pb + 1) * F] = (
                bf32[:, hb * F : (hb + 1) * F]
            )
        pp2[sl, _C_BD1 + pb * F : _C_BD1 + (pb + 1) * F] = 1.0
    return {"pp1": pp1, "pp2": pp2}


def kernel(features, W_feat, b_feat, W_gate, b_gate):
    params = _prep_params(W_feat, b_feat, W_gate, b_gate)
    featf = np.ascontiguousarray(np.asarray(features, np.float32).reshape(N, F))
    nc = _get_nc()
    in_maps = [
        {"feat": featf[i * NC_ROWS : (i + 1) * NC_ROWS], **params}
        for i in range(NCORES)
    ]
    res = run_bass_kernel_spmd(nc, in_maps, list(range(NCORES))).results
    out = np.concatenate([res[i]["out"] for i in range(NCORES)], axis=0)
    return out.reshape(B, S, H)


# revision 25
# speedup vs baseline: 1.0868x; 1.0378x over previous
"""VariableSelectionNetwork Trainium2 kernel (8-core data parallel).

Reference computation per row n (F=32 features, H=64 hidden):
    t[n,f,h] = feat[n,f]*W_feat[f,h] + b_feat[f,h]
    gates    = softmax(t.flat @ W_gate + b_gate)        # over f
    out[n,h] = sum_f t[n,f,h] * gates[n,f]

Algebraic collapse (exact, just reassociated):
    logits = feat @ A + c        A[f,g] = sum_h W_feat[f,h]*W_gate[f*H+h,g]
                                 c      = b_feat.flat @ W_gate + b_gate
    e      = exp(logits)         (logits are O(1); no max-shift needed)
    gates  = e * (1 / sum_f e)
    out    = (feat*gates) @ W_feat + gates @ b_feat

Device dataflow per core (2048 rows), "blocked transpose" scheme, 2 slabs:
    ftile[p, (rb f)] <- DMA          (row = p*16+rb; contiguous per partition)
    featT = StreamTranspose(ftile)   -> featT[(pb f), (rb pl)], row=(pb*32+pl)*16+rb
    lg    = blockdiag(A,x4).T @ featT            (one matmul per slab)
    et    = exp(lg + c_rep)                      (ACT, bias per partition)
    dsum  = blockdiag(ones32,x4).T @ et          -> [4, n] row sums over f
    rcp   = reciprocal_approx_fast(dsum)         (DVE custom op)
    rcp_b = gpsimd partition_broadcast per pb    -> [128, n]
    gatesT= et * rcp_b ; wgfT = featT * gatesT   (DVE)
    outT_hb = blockdiag(W_feat[:,hb],x4).T @ wgfT + blockdiag(b_feat[:,hb],x4).T @ gatesT
    otile[p, (rb h)] = StreamTranspose(outT)     -> DMA out
"""

import sys

sys.path.insert(0, "/opt/trn_rl_repo")

import numpy as np

from concourse import bacc, mybir, tile
from concourse.bass_utils import run_bass_kernel_spmd

B, S, F, H = 32, 512, 32, 64
N = B * S
NCORES = 8
NC_ROWS = N // NCORES  # 2048 rows per core
P = 128
NPB = P // F           # 4 partition blocks
RPP = NC_ROWS // P     # 16 rows per partition
NS = 2                 # slabs (pipeline stages over rb)
RBS = RPP // NS        # rb per slab
NW = RBS * F           # transposed-domain columns per slab
F32 = mybir.dt.float32
F32R = mybir.dt.float32r
EXP = mybir.ActivationFunctionType.Exp
MMDT = F32R            # dtype for matmul operands (f32r: 1-pass PE at N>=256)

# packed param columns: pp1 = [bdA | crep] (needed first), pp2 = rest
PW1 = P + 1
PW2 = 5 * P
_C_BDW = [0, 2 * P]
_C_BDB = [P, 3 * P]
_C_BD1 = 4 * P

_NC_CACHE = {}


def _build_nc():
    nc = bacc.Bacc("TRN2", target_bir_lowering=False, debug=False, num_devices=NCORES)

    feat_d = nc.dram_tensor("feat", [NC_ROWS, F], F32, kind="ExternalInput").ap()
    pp1_d = nc.dram_tensor("pp1", [P, PW1], MMDT, kind="ExternalInput").ap()
    pp2_d = nc.dram_tensor("pp2", [P, PW2], MMDT, kind="ExternalInput").ap()
    out_d = nc.dram_tensor("out", [NC_ROWS, H], F32, kind="ExternalOutput").ap()

    with tile.TileContext(nc) as tc:
        with (
            tc.tile_pool(name="const", bufs=1) as cpool,
            tc.tile_pool(name="work", bufs=2) as work,
            tc.tile_pool(name="ps", bufs=2, space="PSUM") as ps,
        ):
            feat_r0 = feat_d.rearrange("(p r) f -> p (r f)", p=P)
            fbig = work.tile([P, NS * NW], F32, tag="fin")
            nc.sync.dma_start(fbig[:, 0:NW], feat_r0[:, 0:NW])
            nc.scalar.dma_start(fbig[:, NW : 2 * NW], feat_r0[:, NW : 2 * NW])
            ftiles = [fbig[:, s * NW : (s + 1) * NW] for s in range(NS)]

            pp1 = cpool.tile([P, PW1], MMDT)
            nc.gpsimd.dma_start(pp1[:], pp1_d)
            pp2 = cpool.tile([P, PW2], MMDT)
            nc.gpsimd.dma_start(pp2[:], pp2_d)
            bda = pp1[:, 0:P]
            crep = pp1[:, P : P + 1]
            bdw = [pp2[:, c : c + P] for c in _C_BDW]
            bdb = [pp2[:, c : c + P] for c in _C_BDB]
            bd1x = pp2[:, _C_BD1 : _C_BD1 + P]

            feat_r = feat_d.rearrange("(p r) f -> p (r f)", p=P)   # [128, 512]
            out_r = out_d.rearrange("(p r) h -> p (r h)", p=P)     # [128, 1024]

            from copy import deepcopy

            for s in range(NS):
                st_eng = nc.scalar if s % 2 == 0 else nc.sync
                featT = work.tile([P, NW], F32, tag="featT")
                nc.vector.transpose(featT[:], ftiles[s])
                featTr = work.tile([P, NW], MMDT, tag="featTr")
                nc.scalar.copy(featTr[:], featT[:])

                lg = ps.tile([P, NW], F32, tag="lg")
                nc.tensor.matmul(lg[:], bda, featTr[:])

                et = work.tile([P, NW], MMDT, tag="et")
                nc.scalar.activation(et[:], lg[:], EXP, bias=crep)

                # unnormalized: wef = featT*e feeds the out matmuls directly;
                # the reciprocal runs in parallel off the critical chain
                dsum_b = ps.tile([P, NW], F32, tag="dsum")
                nc.tensor.matmul(dsum_b[:], bd1x, et[:])
                rcp_b = work.tile([P, NW], F32, tag="rcpb")
                nc.vector.reciprocal_approx_fast(rcp_b[:], dsum_b[:])

                wef = work.tile([P, NW], MMDT, tag="wef")
                nc.vector.tensor_mul(wef[:], featTr[:], et[:])

                outT = ps.tile([P, 2 * NW], F32, tag="outT")
                for hb in range(2):
                    osl = outT[:, hb * NW : (hb + 1) * NW]
                    nc.tensor.matmul(osl, bdw[hb], wef[:], start=True, stop=False)
                    nc.tensor.matmul(osl, bdb[hb], et[:], start=False, stop=True)

                # normalize both h-halves with one broadcast mul
                outTn = work.tile([P, 2 * NW], F32, tag="outTn")
                rcp_bc = rcp_b[:].rearrange("p (o n) -> p o n", o=1)
                rcp_bc = deepcopy(rcp_bc)
                rcp_bc.ap[1] = [0, 2]
                nc.vector.tensor_mul(
                    outTn[:].rearrange("p (o n) -> p o n", o=2),
                    outT[:].rearrange("p (o n) -> p o n", o=2),
                    rcp_bc,
                )

                otile = work.tile([P, RBS * H], F32, tag="oout")
                otile_blk = otile[:].rearrange("p (rb z) -> p rb z", rb=RBS)
                for hb in range(2):
                    nc.vector.transpose(
                        otile_blk[:, :, hb * F : (hb + 1) * F],
                        outTn[:, hb * NW : (hb + 1) * NW],
                    )
                st_eng.dma_start(
                    out_r[:, s * RBS * H : (s + 1) * RBS * H], otile[:]
                )

    nc.compile()
    return nc


def _get_nc():
    if "nc" not in _NC_CACHE:
        _NC_CACHE["nc"] = _build_nc()
    return _NC_CACHE["nc"]


def _prep_params(W_feat, b_feat, W_gate, b_gate):
    wf = np.asarray(W_feat, np.float64)
    wg = np.asarray(W_gate, np.float64).reshape(F, H, F)
    a = np.einsum("fh,fhg->fg", wf, wg).astype(np.float32)
    c = (
        np.asarray(b_feat, np.float64).reshape(-1) @ np.asarray(W_gate, np.float64)
        + np.asarray(b_gate, np.float64)
    ).astype(np.float32)
    wf32 = np.asarray(W_feat, np.float32)
    bf32 = np.asarray(b_feat, np.float32)

    pp1 = np.zeros((P, PW1), np.float32)
    pp2 = np.zeros((P, PW2), np.float32)
    for pb in range(NPB):
        sl = slice(pb * F, (pb + 1) * F)
        pp1[sl, pb * F : (pb + 1) * F] = a
        pp1[sl, P] = c
        for hb in range(2):
            pp2[sl, _C_BDW[hb] + pb * F : _C_BDW[hb] + (pb + 1) * F] = (
                wf32[:, hb * F : (hb + 1) * F]
            )
            pp2[sl, _C_BDB[hb] + pb * F : _C_BDB[hb] + (pb + 1) * F] = (
                bf32[:, hb * F : (hb + 1) * F]
            )
        pp2[sl, _C_BD1 + pb * F : _C_BD1 + (pb + 1) * F] = 1.0
    return {"pp1": pp1, "pp2": pp2}


def kernel(features, W_feat, b_feat, W_gate, b_gate):
    params = _prep_params(W_feat, b_feat, W_gate, b_gate)
    featf = np.ascontiguousarray(np.asarray(features, np.float32).reshape(N, F))
    nc = _get_nc()
    in_maps = [
        {"feat": featf[i * NC_ROWS : (i + 1) * NC_ROWS], **params}
        for i in range(NCORES)
    ]
    res = run_bass_kernel_spmd(nc, in_maps, list(range(NCORES))).results
    out = np.concatenate([res[i]["out"] for i in range(NCORES)], axis=0)
    return out.reshape(B, S, H)


# revision 26
# speedup vs baseline: 1.1192x; 1.0299x over previous
"""VariableSelectionNetwork Trainium2 kernel (8-core data parallel).

Reference computation per row n (F=32 features, H=64 hidden):
    t[n,f,h] = feat[n,f]*W_feat[f,h] + b_feat[f,h]
    gates    = softmax(t.flat @ W_gate + b_gate)        # over f
    out[n,h] = sum_f t[n,f,h] * gates[n,f]

Algebraic collapse (exact, just reassociated):
    logits = feat @ A + c        A[f,g] = sum_h W_feat[f,h]*W_gate[f*H+h,g]
                                 c      = b_feat.flat @ W_gate + b_gate
    e      = exp(logits)         (logits are O(1); no max-shift needed)
    gates  = e * (1 / sum_f e)
    out    = (feat*gates) @ W_feat + gates @ b_feat

Device dataflow per core (2048 rows), "blocked transpose" scheme, 2 slabs:
    ftile[p, (rb f)] <- DMA          (row = p*16+rb; contiguous per partition)
    featT = StreamTranspose(ftile)   -> featT[(pb f), (rb pl)], row=(pb*32+pl)*16+rb
    lg    = blockdiag(A,x4).T @ featT            (one matmul per slab)
    et    = exp(lg + c_rep)                      (ACT, bias per partition)
    dsum  = blockdiag(ones32,x4).T @ et          -> [4, n] row sums over f
    rcp   = reciprocal_approx_fast(dsum)         (DVE custom op)
    rcp_b = gpsimd partition_broadcast per pb    -> [128, n]
    gatesT= et * rcp_b ; wgfT = featT * gatesT   (DVE)
    outT_hb = blockdiag(W_feat[:,hb],x4).T @ wgfT + blockdiag(b_feat[:,hb],x4).T @ gatesT
    otile[p, (rb h)] = StreamTranspose(outT)     -> DMA out
"""

import sys

sys.path.insert(0, "/opt/trn_rl_repo")

import numpy as np

from concourse import bacc, mybir, tile
from concourse.bass_utils import run_bass_kernel_spmd

B, S, F, H = 32, 512, 32, 64
N = B * S
NCORES = 8
NC_ROWS = N // NCORES  # 2048 rows per core
P = 128
NPB = P // F           # 4 partition blocks
RPP = NC_ROWS // P     # 16 rows per partition
NS = 2                 # slabs (pipeline stages over rb)
RBS = RPP // NS        # rb per slab
NW = RBS * F           # transposed-domain columns per slab
F32 = mybir.dt.float32
F32R = mybir.dt.float32r
EXP = mybir.ActivationFunctionType.Exp
MMDT = mybir.dt.float16  # dtype for matmul operands

# packed param columns: pp1 = [bdA | crep] (needed first), pp2 = rest
PW1 = P + 1
PW2 = 5 * P
_C_BDW = [0, 2 * P]
_C_BDB = [P, 3 * P]
_C_BD1 = 4 * P

_NC_CACHE = {}


def _build_nc():
    nc = bacc.Bacc("TRN2", target_bir_lowering=False, debug=False, num_devices=NCORES)

    feat_d = nc.dram_tensor("feat", [NC_ROWS, F], F32, kind="ExternalInput").ap()
    pp1_d = nc.dram_tensor("pp1", [P, PW1], MMDT, kind="ExternalInput").ap()
    pp2_d = nc.dram_tensor("pp2", [P, PW2], MMDT, kind="ExternalInput").ap()
    out_d = nc.dram_tensor("out", [NC_ROWS, H], F32, kind="ExternalOutput").ap()

    with tile.TileContext(nc) as tc:
        with (
            tc.tile_pool(name="const", bufs=1) as cpool,
            tc.tile_pool(name="work", bufs=2) as work,
            tc.tile_pool(name="ps", bufs=2, space="PSUM") as ps,
        ):
            feat_r0 = feat_d.rearrange("(p r) f -> p (r f)", p=P)
            fbig = work.tile([P, NS * NW], F32, tag="fin")
            nc.sync.dma_start(fbig[:, 0:NW], feat_r0[:, 0:NW])
            nc.scalar.dma_start(fbig[:, NW : 2 * NW], feat_r0[:, NW : 2 * NW])
            ftiles = [fbig[:, s * NW : (s + 1) * NW] for s in range(NS)]

            pp1 = cpool.tile([P, PW1], MMDT)
            nc.gpsimd.dma_start(pp1[:], pp1_d)
            pp2 = cpool.tile([P, PW2], MMDT)
            nc.gpsimd.dma_start(pp2[:], pp2_d)
            bda = pp1[:, 0:P]
            crep = pp1[:, P : P + 1]
            bdw = [pp2[:, c : c + P] for c in _C_BDW]
            bdb = [pp2[:, c : c + P] for c in _C_BDB]
            bd1x = pp2[:, _C_BD1 : _C_BD1 + P]

            feat_r = feat_d.rearrange("(p r) f -> p (r f)", p=P)   # [128, 512]
            out_r = out_d.rearrange("(p r) h -> p (r h)", p=P)     # [128, 1024]

            from copy import deepcopy

            for s in range(NS):
                st_eng = nc.scalar if s % 2 == 0 else nc.sync
                featT = work.tile([P, NW], F32, tag="featT")
                nc.vector.transpose(featT[:], ftiles[s])
                featTr = work.tile([P, NW], MMDT, tag="featTr")
                nc.scalar.copy(featTr[:], featT[:])

                lg = ps.tile([P, NW], F32, tag="lg")
                nc.tensor.matmul(lg[:], bda, featTr[:])

                et = work.tile([P, NW], MMDT, tag="et")
                nc.scalar.activation(et[:], lg[:], EXP, bias=crep)

                # unnormalized: wef = featT*e feeds the out matmuls directly;
                # the reciprocal runs in parallel off the critical chain
                dsum_b = ps.tile([P, NW], F32, tag="dsum")
                nc.tensor.matmul(dsum_b[:], bd1x, et[:])
                rcp_b = work.tile([P, NW], F32, tag="rcpb")
                nc.vector.reciprocal_approx_fast(rcp_b[:], dsum_b[:])

                wef = work.tile([P, NW], MMDT, tag="wef")
                nc.vector.tensor_mul(wef[:], featTr[:], et[:])

                outT = ps.tile([P, 2 * NW], F32, tag="outT")
                for hb in range(2):
                    osl = outT[:, hb * NW : (hb + 1) * NW]
                    nc.tensor.matmul(osl, bdw[hb], wef[:], start=True, stop=False)
                    nc.tensor.matmul(osl, bdb[hb], et[:], start=False, stop=True)

                # normalize both h-halves with one broadcast mul
                outTn = work.tile([P, 2 * NW], F32, tag="outTn")
                rcp_bc = rcp_b[:].rearrange("p (o n) -> p o n", o=1)
                rcp_bc = deepcopy(rcp_bc)
                rcp_bc.ap[1] = [0, 2]
                nc.vector.tensor_mul(
                    outTn[:].rearrange("p (o n) -> p o n", o=2),
                    outT[:].rearrange("p (o n) -> p o n", o=2),
                    rcp_bc,
                )

                otile = work.tile([P, RBS * H], F32, tag="oout")
                otile_blk = otile[:].rearrange("p (rb z) -> p rb z", rb=RBS)
                for hb in range(2):
                    nc.vector.transpose(
                        otile_blk[:, :, hb * F : (hb + 1) * F],
                        outTn[:, hb * NW : (hb + 1) * NW],
                    )
                st_eng.dma_start(
                    out_r[:, s * RBS * H : (s + 1) * RBS * H], otile[:]
                )

    nc.compile()
    return nc


def _get_nc():
    if "nc" not in _NC_CACHE:
        _NC_CACHE["nc"] = _build_nc()
    return _NC_CACHE["nc"]


def _prep_params(W_feat, b_feat, W_gate, b_gate):
    wf = np.asarray(W_feat, np.float64)
    wg = np.asarray(W_gate, np.float64).reshape(F, H, F)
    a = np.einsum("fh,fhg->fg", wf, wg).astype(np.float32)
    c = (
        np.asarray(b_feat, np.float64).reshape(-1) @ np.asarray(W_gate, np.float64)
        + np.asarray(b_gate, np.float64)
    ).astype(np.float32)
    wf32 = np.asarray(W_feat, np.float32)
    bf32 = np.asarray(b_feat, np.float32)

    pp1 = np.zeros((P, PW1), np.float16)
    pp2 = np.zeros((P, PW2), np.float16)
    for pb in range(NPB):
        sl = slice(pb * F, (pb + 1) * F)
        pp1[sl, pb * F : (pb + 1) * F] = a
        pp1[sl, P] = c
        for hb in range(2):
            pp2[sl, _C_BDW[hb] + pb * F : _C_BDW[hb] + (pb + 1) * F] = (
                wf32[:, hb * F : (hb + 1) * F]
            )
            pp2[sl, _C_BDB[hb] + pb * F : _C_BDB[hb] + (pb + 1) * F] = (
                bf32[:, hb * F : (hb + 1) * F]
            )
        pp2[sl, _C_BD1 + pb * F : _C_BD1 + (pb + 1) * F] = 1.0
    return {"pp1": pp1, "pp2": pp2}


def kernel(features, W_feat, b_feat, W_gate, b_gate):
    params = _prep_params(W_feat, b_feat, W_gate, b_gate)
    featf = np.ascontiguousarray(np.asarray(features, np.float32).reshape(N, F))
    nc = _get_nc()
    in_maps = [
        {"feat": featf[i * NC_ROWS : (i + 1) * NC_ROWS], **params}
        for i in range(NCORES)
    ]
    res = run_bass_kernel_spmd(nc, in_maps, list(range(NCORES))).results
    out = np.concatenate([res[i]["out"] for i in range(NCORES)], axis=0)
    return out.reshape(B, S, H)
